# revision 20
# baseline (speedup 1.0000x reference)
"""Trainium2 Bass kernel for nn_AttributeBranch (MCB attribute pooling).

Algorithm note
--------------
In the reference, the "attr" operand of MCB is spatially constant
(emb[b, c] broadcast over H, W).  The MCB circular convolution with a
spatially-constant sketch therefore collapses to a per-batch circulant
matrix C_b[n, d] = sx_b[(n - d) mod 256], where sx_b is the count-sketch
of the embedding.  The whole network becomes

    hidden_b  = relu( (conv1_w @ C_b @ E2) @ y_b )        y_b = entity [256, 784]
    attr_map  = sigmoid(conv2_w @ hidden_b)
    attr_feat = attr_map * y_b

with E2[d, c] = s2[c] * (h2[c] == d) the dense count-sketch matrix.
Per batch we compute MT_b = (conv1_w @ C_b @ E2)^T  =  E2^T @ C_b^T @ conv1_w^T
via two tiny matmuls (the circulant tile is materialized by a strided DMA
from a periodic repeated-reversed copy of sx_b), then one
[32,256]x[256,784] matmul, conv2 + sigmoid, and a broadcast-multiply
against the entity tile.

Matmuls run in float32r (TF32-class, 1 cycle/row at N>=256, ~2e-4 rel err);
the final elementwise multiply and all accumulations stay fp32.

Sharding: pure data-parallel, batch 64 -> 8 cores x 8.
DMA queues: nc.sync (qSp HWDGE) carries loads, nc.scalar (qAct HWDGE)
carries stores + the circulant scratch write.
"""

import sys

for _p in ("/opt/trn_rl_repo",):
    if _p not in sys.path:
        sys.path.insert(0, _p)

from contextlib import ExitStack

import numpy as np

import concourse.bacc as bacc
import concourse.bass as bass
import concourse.mybir as mybir
import concourse.tile as tile
from concourse.ap import AP
from concourse.bass_utils import run_bass_kernel_spmd

NCORES = 8
B, C, H, W = 64, 256, 28, 28
HW = H * W
D = 256          # sketch dimension
BL = B // NCORES # batches per core
KPAD = 384       # attr dim 300 padded to 3x128

f32 = mybir.dt.float32
f32r = mybir.dt.float32r
AF = mybir.ActivationFunctionType


def _build_program():
    nc = bacc.Bacc("TRN2", target_bir_lowering=False, debug=False)

    ent_d = nc.dram_tensor("ent", [BL, C, HW], f32r, kind="ExternalInput")
    aohT_d = nc.dram_tensor("aohT", [KPAD, BL], f32r, kind="ExternalInput")
    wembT_d = nc.dram_tensor("wembT", [KPAD, C], f32r, kind="ExternalInput")
    bemb_d = nc.dram_tensor("bemb", [C, 1], f32, kind="ExternalInput")
    d1_d = nc.dram_tensor("d1", [C, D], f32r, kind="ExternalInput")
    e2_d = nc.dram_tensor("e2", [D, C], f32r, kind="ExternalInput")
    cwT_d = nc.dram_tensor("cwT", [C, 32], f32r, kind="ExternalInput")
    c2wT_d = nc.dram_tensor("c2wT", [32, 1], f32r, kind="ExternalInput")

    map_d = nc.dram_tensor("attr_map", [BL, HW], f32r, kind="ExternalOutput")
    af_d = nc.dram_tensor("attr_feature", [BL, C, HW], f32, kind="ExternalOutput")

    # Periodic repeated-reversed sketch rows, bounced through DRAM so the
    # circulant tiles can be materialized with a strided DMA read.  Row
    # layout: st_b[y] = sx_b[(-y) mod 256], repeated 129x; a read with
    # partition step +255 (= -1 mod 256) and free step +1 then yields
    # C[n, d] = sx[(n - d) mod 256].  (Negative DMA partition steps are
    # rejected by the BIR verifier, hence the +255 periodic trick.)
    NREP = 129
    SLEN = NREP * D
    rscr = nc.dram_tensor("rscr", [BL, SLEN], f32r)

    with tile.TileContext(nc) as tc, ExitStack() as ctx:
        const = ctx.enter_context(tc.tile_pool(name="const", bufs=1))
        work = ctx.enter_context(tc.tile_pool(name="work", bufs=4))
        entp = ctx.enter_context(tc.tile_pool(name="entp", bufs=16))
        circp = ctx.enter_context(tc.tile_pool(name="circp", bufs=6))
        afp = ctx.enter_context(tc.tile_pool(name="afp", bufs=4))
        psS = ctx.enter_context(tc.tile_pool(name="psS", bufs=4, space="PSUM"))
        psB = ctx.enter_context(tc.tile_pool(name="psB", bufs=2, space="PSUM"))

        def load_const(name, dram, nblk, cols, dt=f32r, eng=None):
            eng = eng or nc.sync
            ts = []
            for k in range(nblk):
                rows = min(128, dram.shape[0] - k * 128)
                t = const.tile([rows, cols], dt, tag=f"{name}{k}")
                eng.dma_start(t[:], dram.ap()[k * 128:k * 128 + rows, :])
                ts.append(t)
            return ts

        # emb-path constants first (they gate the circulant scratch write)
        wembT = load_const("wembT", wembT_d, 3, C)
        aohT = load_const("aohT", aohT_d, 3, BL)
        bemb = load_const("bemb", bemb_d, 2, 1, dt=f32)
        d1 = load_const("d1", d1_d, 2, D)
        # second-group constants on the store queue (idle during prologue)
        e2 = load_const("e2", e2_d, 2, C, eng=nc.scalar)
        cwT = load_const("cwT", cwT_d, 2, 32, eng=nc.scalar)
        c2wT = load_const("c2wT", c2wT_d, 1, 1, eng=nc.scalar)[0]
        ones_f = const.tile([1, 128], f32, tag="ones_f")
        nc.vector.memset(ones_f[:], 1.0)
        ones = const.tile([1, 128], f32r, tag="ones")
        nc.vector.tensor_copy(ones[:], ones_f[:])

        # ---- embT[c, b] = (W_emb @ aoh^T + b_emb) ----------------------
        embT = []
        for mb in range(2):
            pe = psS.tile([128, BL], f32, tag="ps_small")
            for k in range(3):
                nc.tensor.matmul(pe[:], wembT[k][:, mb * 128:(mb + 1) * 128],
                                 aohT[k][:], start=(k == 0), stop=(k == 2))
            t = work.tile([128, BL], f32r, tag=f"embT{mb}")
            nc.vector.tensor_scalar_add(t[:], pe[:], bemb[mb][:])
            embT.append(t)

        # ---- sx[b, d] = (emb @ D1) ------------------------------------
        psx = psS.tile([BL, D], f32, tag="ps_small")
        for kb in range(2):
            nc.tensor.matmul(psx[:], embT[kb][:], d1[kb][:],
                             start=(kb == 0), stop=(kb == 1))
        # crow[b, y] = sx[b, (-y) mod 256], read straight out of PSUM
        crow = work.tile([BL, D], f32r, tag="crow")
        nc.vector.tensor_copy(crow[:, 0:1], psx[:, 0:1])
        nc.vector.tensor_copy(crow[:, 1:D], psx[:, 1:D][:, ::-1])
        # Stage the rows in DRAM, then replicate DRAM->DRAM per batch.
        # (An SBUF-sourced broadcast write re-reads each 1KB partition row
        # 129x through one SBUF port -- ~25us stall.  DRAM-sourced
        # replication runs at queue bandwidth.)
        crow_d = nc.dram_tensor("crow_d", [BL, D], f32r)
        cw_st = nc.scalar.dma_start(crow_d.ap()[:], crow[:])
        reps = []
        for b in range(BL):
            eng = nc.scalar if b % 2 == 0 else nc.sync
            r = eng.dma_start(
                AP(rscr, b * SLEN, [[D, NREP], [1, D]]),
                AP(crow_d, b * D, [[0, NREP], [1, D]]))
            tile.add_dep_helper(r.ins, cw_st.ins, reason="crow_d RAW")
            reps.append(r)

        # ---- phase 1: circulants + V for all batches ------------------
        # V_b[d, o] = sum_n C_b[n, d] * conv1_w[o, n], stacked along the
        # free dim into V_all[d, b*32+o] so the E2 contraction can run
        # once for all batches at N=256.
        v_all = [const.tile([128, BL * 32], f32r, tag=f"vall{db}",
                            name=f"vall{db}")
                 for db in range(2)]
        ent_ts = []
        for b in range(BL):
            # circulant tiles C[n, d] = sx[(n - d) % 256] = st_b[A + 255*n + d]
            # block 0: A = 0; block 1 (n = 128 + p): A = 128 (== -128 mod 256)
            circ = []
            circ_lds = []
            for nb in range(2):
                t = circp.tile([128, D], f32r, tag="circ")
                src = AP(rscr, b * SLEN + 128 * nb, [[255, 128], [1, D]])
                ld = nc.sync.dma_start(t[:], src)
                tile.add_dep_helper(ld.ins, reps[b].ins, reason="rscr RAW")
                circ.append(t)
                circ_lds.append(ld)

            ent_t = []
            for cb in range(2):
                t = entp.tile([128, HW], f32r, tag="ent")
                eld = (nc.sync if cb == 0 else nc.scalar).dma_start(
                    t[:], ent_d.ap()[b, cb * 128:(cb + 1) * 128, :])
                if b >= 2:
                    # pace the prefetch: keep the queue FIFOs from running
                    # ahead of the latency-critical circulant chain
                    tile.add_dep_helper(eld.ins, circ_lds[0].ins,
                                        reason="prefetch pacing", sync=False)
                ent_t.append(t)
            ent_ts.append(ent_t)

            for db in range(2):
                pv = psS.tile([128, 32], f32, tag="ps_small")
                for nb in range(2):
                    nc.tensor.matmul(pv[:], circ[nb][:, db * 128:(db + 1) * 128],
                                     cwT[nb][:], start=(nb == 0), stop=(nb == 1))
                nc.vector.tensor_copy(v_all[db][:, b * 32:(b + 1) * 32], pv[:])

        # ---- MT_all[c, (b,o)] = sum_d E2[d, c] * V_all[d, (b,o)] ------
        mtall = []
        for cb in range(2):
            pm = psS.tile([128, BL * 32], f32, tag="ps_small")
            for db in range(2):
                nc.tensor.matmul(pm[:], e2[db][:, cb * 128:(cb + 1) * 128],
                                 v_all[db][:], start=(db == 0), stop=(db == 1))
            t = const.tile([128, BL * 32], f32r, tag=f"mtall{cb}",
                           name=f"mtall{cb}")
            nc.vector.tensor_copy(t[:], pm[:])
            mtall.append(t)

        # ---- phase 2: per-batch hidden / map / attr_feature -----------
        NSPLIT = [(0, 512), (512, HW - 512)]
        for b in range(BL):
            ent_t = ent_ts[b]

            # hidden[o, hw] = relu(sum_c MT[c, o] * ent[c, hw])
            ph = psB.tile([32, HW], f32, tag="ps_big")
            for n0, nn in NSPLIT:
                for cb in range(2):
                    nc.tensor.matmul(ph[:, n0:n0 + nn],
                                     mtall[cb][:, b * 32:(b + 1) * 32],
                                     ent_t[cb][:, n0:n0 + nn],
                                     start=(cb == 0), stop=(cb == 1))
            hid = work.tile([32, HW], f32r, tag="hid")
            nc.scalar.activation(hid[:], ph[:], AF.Relu)

            # attr_map[hw] = sigmoid(sum_o conv2_w[0, o] * hidden[o, hw])
            pmap = psB.tile([1, HW], f32, tag="ps_big")
            for n0, nn in NSPLIT:
                nc.tensor.matmul(pmap[:, n0:n0 + nn], c2wT[:],
                                 hid[:, n0:n0 + nn], start=True, stop=True)
            map_t = work.tile([1, HW], f32r, tag="map")
            nc.scalar.activation(map_t[:], pmap[:], AF.Sigmoid)
            nc.scalar.dma_start(map_d.ap()[b:b + 1, :], map_t[:])

            # broadcast map across 128 partitions on GPSIMD (PE stays free)
            bc = work.tile([128, HW], f32, tag="bc")
            nc.gpsimd.partition_broadcast(bc[:], map_t[:].bitcast(f32))

            for cb in range(2):
                af = afp.tile([128, HW], f32, tag="af")
                nc.vector.tensor_tensor(af[:], ent_t[cb][:].bitcast(f32), bc[:],
                                        op=mybir.AluOpType.mult)
                (nc.scalar if cb == 0 else nc.sync).dma_start(
                    af_d.ap()[b, cb * 128:(cb + 1) * 128, :], af[:])

    nc.compile()
    return nc


_NC = None


def _get_nc():
    global _NC
    if _NC is None:
        _NC = _build_program()
    return _NC


def _make_in_maps(inputs):
    ent = np.ascontiguousarray(inputs["entity_feature"], np.float32).reshape(B, C, HW)
    aoh = np.asarray(inputs["attr_one_hot"], np.float32)
    Wemb = np.asarray(inputs["W_emb"], np.float32)
    bemb = np.asarray(inputs["b_emb"], np.float32).reshape(C, 1)
    h1 = np.asarray(inputs["h1"]).astype(np.int64)
    s1 = np.asarray(inputs["s1"], np.float32)
    h2 = np.asarray(inputs["h2"]).astype(np.int64)
    s2 = np.asarray(inputs["s2"], np.float32)
    cw = np.asarray(inputs["conv1_w"], np.float32)
    c2w = np.asarray(inputs["conv2_w"], np.float32)

    # dense count-sketch matrices (pure re-indexing of h/s inputs)
    D1 = np.zeros((C, D), np.float32)
    D1[np.arange(C), h1] = s1
    E2 = np.zeros((D, C), np.float32)
    E2[h2, np.arange(C)] = s2

    wembT = np.zeros((KPAD, C), np.float32)
    wembT[:Wemb.shape[1]] = Wemb.T
    cwT = np.ascontiguousarray(cw.T)
    c2wT = np.ascontiguousarray(c2w.T)

    in_maps = []
    for c in range(NCORES):
        sl = slice(c * BL, (c + 1) * BL)
        aohT = np.zeros((KPAD, BL), np.float32)
        aohT[:aoh.shape[1]] = aoh[sl].T
        in_maps.append({
            "ent": np.ascontiguousarray(ent[sl]),
            "aohT": aohT,
            "wembT": wembT,
            "bemb": bemb,
            "d1": D1,
            "e2": E2,
            "cwT": cwT,
            "c2wT": c2wT,
        })
    return in_maps


def run_on_hw(inputs, trace=False, trace_cores=None):
    nc = _get_nc()
    in_maps = _make_in_maps(inputs)
    br = run_bass_kernel_spmd(
        nc, in_maps, list(range(NCORES)), trace=trace,
        trace_cores=trace_cores,
    )
    maps = np.concatenate([r["attr_map"] for r in br.results], axis=0)
    feats = np.concatenate([r["attr_feature"] for r in br.results], axis=0)
    attr_map = maps.reshape(B, 1, H, W).astype(np.float32)
    attr_feature = feats.reshape(B, C, H, W).astype(np.float32)
    return (attr_map, attr_feature), br


def kernel(**inputs):
    (attr_map, attr_feature), _ = run_on_hw(inputs, trace=False)
    return attr_map, attr_feature


# revision 21
# speedup vs baseline: 1.1088x; 1.1088x over previous
"""Trainium2 Bass kernel for nn_AttributeBranch (MCB attribute pooling).

Algorithm note
--------------
In the reference, the "attr" operand of MCB is spatially constant
(emb[b, c] broadcast over H, W).  The MCB circular convolution with a
spatially-constant sketch therefore collapses to a per-batch circulant
matrix C_b[n, d] = sx_b[(n - d) mod 256], where sx_b is the count-sketch
of the embedding.  The whole network becomes

    hidden_b  = relu( (conv1_w @ C_b @ E2) @ y_b )        y_b = entity [256, 784]
    attr_map  = sigmoid(conv2_w @ hidden_b)
    attr_feat = attr_map * y_b

with E2[d, c] = s2[c] * (h2[c] == d) the dense count-sketch matrix.
Per batch we compute MT_b = (conv1_w @ C_b @ E2)^T  =  E2^T @ C_b^T @ conv1_w^T
via two tiny matmuls (the circulant tile is materialized by a strided DMA
from a periodic repeated-reversed copy of sx_b), then one
[32,256]x[256,784] matmul, conv2 + sigmoid, and a broadcast-multiply
against the entity tile.

Matmuls run in float32r (TF32-class, 1 cycle/row at N>=256, ~2e-4 rel err);
the final elementwise multiply and all accumulations stay fp32.

Sharding: pure data-parallel, batch 64 -> 8 cores x 8.
DMA queues: nc.sync (qSp HWDGE) carries loads, nc.scalar (qAct HWDGE)
carries stores + the circulant scratch write.
"""

import sys

for _p in ("/opt/trn_rl_repo",):
    if _p not in sys.path:
        sys.path.insert(0, _p)

from contextlib import ExitStack

import numpy as np

import concourse.bacc as bacc
import concourse.bass as bass
import concourse.mybir as mybir
import concourse.tile as tile
from concourse.ap import AP
from concourse.bass_utils import run_bass_kernel_spmd

NCORES = 8
B, C, H, W = 64, 256, 28, 28
HW = H * W
D = 256          # sketch dimension
BL = B // NCORES # batches per core
KPAD = 384       # attr dim 300 padded to 3x128

f32 = mybir.dt.float32
f32r = mybir.dt.float32r
AF = mybir.ActivationFunctionType


def _build_program():
    nc = bacc.Bacc("TRN2", target_bir_lowering=False, debug=False)

    ent_d = nc.dram_tensor("ent", [BL, C, HW], f32r, kind="ExternalInput")
    aohT_d = nc.dram_tensor("aohT", [KPAD, BL], f32r, kind="ExternalInput")
    wembT_d = nc.dram_tensor("wembT", [KPAD, C], f32r, kind="ExternalInput")
    bemb_d = nc.dram_tensor("bemb", [C, 1], f32, kind="ExternalInput")
    d1_d = nc.dram_tensor("d1", [C, D], f32r, kind="ExternalInput")
    e2_d = nc.dram_tensor("e2", [D, C], f32r, kind="ExternalInput")
    cwT_d = nc.dram_tensor("cwT", [C, 32], f32r, kind="ExternalInput")
    c2wT_d = nc.dram_tensor("c2wT", [32, 1], f32r, kind="ExternalInput")

    map_d = nc.dram_tensor("attr_map", [BL, HW], f32r, kind="ExternalOutput")
    af_d = nc.dram_tensor("attr_feature", [BL, C, HW], f32, kind="ExternalOutput")

    # Periodic repeated-reversed sketch rows, bounced through DRAM so the
    # circulant tiles can be materialized with a strided DMA read.  Row
    # layout: st_b[y] = sx_b[(-y) mod 256], repeated 129x; a read with
    # partition step +255 (= -1 mod 256) and free step +1 then yields
    # C[n, d] = sx[(n - d) mod 256].  (Negative DMA partition steps are
    # rejected by the BIR verifier, hence the +255 periodic trick.)
    NREP = 65
    SLEN = NREP * D
    rscr = nc.dram_tensor("rscr", [BL, SLEN], f32r)

    with tile.TileContext(nc) as tc, ExitStack() as ctx:
        const = ctx.enter_context(tc.tile_pool(name="const", bufs=1))
        work = ctx.enter_context(tc.tile_pool(name="work", bufs=4))
        entp = ctx.enter_context(tc.tile_pool(name="entp", bufs=16))
        circp = ctx.enter_context(tc.tile_pool(name="circp", bufs=6))
        afp = ctx.enter_context(tc.tile_pool(name="afp", bufs=4))
        psS = ctx.enter_context(tc.tile_pool(name="psS", bufs=4, space="PSUM"))
        psB = ctx.enter_context(tc.tile_pool(name="psB", bufs=2, space="PSUM"))

        def load_const(name, dram, nblk, cols, dt=f32r, eng=None):
            eng = eng or nc.sync
            ts = []
            for k in range(nblk):
                rows = min(128, dram.shape[0] - k * 128)
                t = const.tile([rows, cols], dt, tag=f"{name}{k}")
                eng.dma_start(t[:], dram.ap()[k * 128:k * 128 + rows, :])
                ts.append(t)
            return ts

        # emb-path constants first (they gate the circulant scratch write)
        wembT = load_const("wembT", wembT_d, 3, C)
        aohT = load_const("aohT", aohT_d, 3, BL)
        bemb = load_const("bemb", bemb_d, 2, 1, dt=f32)
        d1 = load_const("d1", d1_d, 2, D)
        # second-group constants on the store queue (idle during prologue)
        e2 = load_const("e2", e2_d, 2, C, eng=nc.scalar)
        cwT = load_const("cwT", cwT_d, 2, 32, eng=nc.scalar)
        c2wT = load_const("c2wT", c2wT_d, 1, 1, eng=nc.scalar)[0]
        ones_f = const.tile([1, 128], f32, tag="ones_f")
        nc.vector.memset(ones_f[:], 1.0)
        ones = const.tile([1, 128], f32r, tag="ones")
        nc.vector.tensor_copy(ones[:], ones_f[:])

        # ---- embT[c, b] = (W_emb @ aoh^T + b_emb) ----------------------
        embT = []
        for mb in range(2):
            pe = psS.tile([128, BL], f32, tag="ps_small")
            for k in range(3):
                nc.tensor.matmul(pe[:], wembT[k][:, mb * 128:(mb + 1) * 128],
                                 aohT[k][:], start=(k == 0), stop=(k == 2))
            t = work.tile([128, BL], f32r, tag=f"embT{mb}")
            nc.vector.tensor_scalar_add(t[:], pe[:], bemb[mb][:])
            embT.append(t)

        # ---- sx[b, d] = (emb @ D1) ------------------------------------
        psx = psS.tile([BL, D], f32, tag="ps_small")
        for kb in range(2):
            nc.tensor.matmul(psx[:], embT[kb][:], d1[kb][:],
                             start=(kb == 0), stop=(kb == 1))
        # crow[b, y] = sx[b, (-y) mod 256], read straight out of PSUM
        crow = work.tile([BL, D], f32r, tag="crow")
        nc.vector.tensor_copy(crow[:, 0:1], psx[:, 0:1])
        nc.vector.tensor_copy(crow[:, 1:D], psx[:, 1:D][:, ::-1])
        # Stage the rows in DRAM, then replicate DRAM->DRAM per batch.
        # (An SBUF-sourced broadcast write re-reads each 1KB partition row
        # 129x through one SBUF port -- ~25us stall.  DRAM-sourced
        # replication runs at queue bandwidth.)
        crow_d = nc.dram_tensor("crow_d", [BL, D], f32r)
        cw_st = nc.scalar.dma_start(crow_d.ap()[:], crow[:])
        reps = []
        for b in range(BL):
            eng = nc.scalar if b % 2 == 0 else nc.sync
            r = eng.dma_start(
                AP(rscr, b * SLEN, [[D, NREP], [1, D]]),
                AP(crow_d, b * D, [[0, NREP], [1, D]]))
            tile.add_dep_helper(r.ins, cw_st.ins, reason="crow_d RAW")
            reps.append(r)

        # ---- per-batch pipeline ---------------------------------------
        NSPLIT = [(0, 512), (512, HW - 512)]
        for b in range(BL):
            # circulant tiles C[n, d] = sx[(n - d) % 256] = st_b[A + 255*n + d]
            # with A == -n0 (mod 256) per 64-partition chunk
            circ = []
            circ_lds = []
            for nb in range(2):
                t = circp.tile([128, D], f32r, tag="circ")
                for half in range(2):
                    n0 = 128 * nb + 64 * half
                    base = (-n0) % 256
                    src = AP(rscr, b * SLEN + base, [[255, 64], [1, D]])
                    ld = nc.sync.dma_start(t[64 * half:64 * half + 64, :], src)
                    tile.add_dep_helper(ld.ins, reps[b].ins, reason="rscr RAW")
                    circ_lds.append(ld)
                circ.append(t)

            ent_t = []
            for cb in range(2):
                t = entp.tile([128, HW], f32r, tag="ent")
                eld = nc.sync.dma_start(
                    t[:], ent_d.ap()[b, cb * 128:(cb + 1) * 128, :])
                if b >= 2:
                    # pace the prefetch: keep the queue FIFOs from running
                    # ahead of the latency-critical circulant chain
                    tile.add_dep_helper(eld.ins, circ_lds[0].ins,
                                        reason="prefetch pacing", sync=False)
                ent_t.append(t)

            # V[d, o] = sum_n C[n, d] * conv1_w[o, n]
            v = []
            for db in range(2):
                pv = psS.tile([128, 32], f32, tag="ps_small")
                for nb in range(2):
                    nc.tensor.matmul(pv[:], circ[nb][:, db * 128:(db + 1) * 128],
                                     cwT[nb][:], start=(nb == 0), stop=(nb == 1))
                t = work.tile([128, 32], f32r, tag="v")
                nc.vector.tensor_copy(t[:], pv[:])
                v.append(t)

            # MT[c, o] = sum_d E2[d, c] * V[d, o]
            mt = []
            for cb in range(2):
                pm = psS.tile([128, 32], f32, tag="ps_small")
                for db in range(2):
                    nc.tensor.matmul(pm[:], e2[db][:, cb * 128:(cb + 1) * 128],
                                     v[db][:], start=(db == 0), stop=(db == 1))
                t = work.tile([128, 32], f32r, tag="mt")
                nc.vector.tensor_copy(t[:], pm[:])
                mt.append(t)

            # hidden[o, hw] = relu(sum_c MT[c, o] * ent[c, hw])
            ph = psB.tile([32, HW], f32, tag="ps_big")
            for n0, nn in NSPLIT:
                for cb in range(2):
                    nc.tensor.matmul(ph[:, n0:n0 + nn], mt[cb][:],
                                     ent_t[cb][:, n0:n0 + nn],
                                     start=(cb == 0), stop=(cb == 1))
            hid = work.tile([32, HW], f32r, tag="hid")
            nc.scalar.activation(hid[:], ph[:], AF.Relu)

            # attr_map[hw] = sigmoid(sum_o conv2_w[0, o] * hidden[o, hw])
            pmap = psB.tile([1, HW], f32, tag="ps_big")
            for n0, nn in NSPLIT:
                nc.tensor.matmul(pmap[:, n0:n0 + nn], c2wT[:],
                                 hid[:, n0:n0 + nn], start=True, stop=True)
            map_t = work.tile([1, HW], f32r, tag="map")
            nc.scalar.activation(map_t[:], pmap[:], AF.Sigmoid)
            nc.scalar.dma_start(map_d.ap()[b:b + 1, :], map_t[:])

            # broadcast map across 128 partitions on GPSIMD (PE stays free)
            bc = work.tile([128, HW], f32, tag="bc")
            nc.gpsimd.partition_broadcast(bc[:], map_t[:].bitcast(f32))

            for cb in range(2):
                af = afp.tile([128, HW], f32, tag="af")
                nc.vector.tensor_tensor(af[:], ent_t[cb][:].bitcast(f32), bc[:],
                                        op=mybir.AluOpType.mult)
                nc.scalar.dma_start(
                    af_d.ap()[b, cb * 128:(cb + 1) * 128, :], af[:])

    nc.compile()
    return nc


_NC = None


def _get_nc():
    global _NC
    if _NC is None:
        _NC = _build_program()
    return _NC


def _make_in_maps(inputs):
    ent = np.ascontiguousarray(inputs["entity_feature"], np.float32).reshape(B, C, HW)
    aoh = np.asarray(inputs["attr_one_hot"], np.float32)
    Wemb = np.asarray(inputs["W_emb"], np.float32)
    bemb = np.asarray(inputs["b_emb"], np.float32).reshape(C, 1)
    h1 = np.asarray(inputs["h1"]).astype(np.int64)
    s1 = np.asarray(inputs["s1"], np.float32)
    h2 = np.asarray(inputs["h2"]).astype(np.int64)
    s2 = np.asarray(inputs["s2"], np.float32)
    cw = np.asarray(inputs["conv1_w"], np.float32)
    c2w = np.asarray(inputs["conv2_w"], np.float32)

    # dense count-sketch matrices (pure re-indexing of h/s inputs)
    D1 = np.zeros((C, D), np.float32)
    D1[np.arange(C), h1] = s1
    E2 = np.zeros((D, C), np.float32)
    E2[h2, np.arange(C)] = s2

    wembT = np.zeros((KPAD, C), np.float32)
    wembT[:Wemb.shape[1]] = Wemb.T
    cwT = np.ascontiguousarray(cw.T)
    c2wT = np.ascontiguousarray(c2w.T)

    in_maps = []
    for c in range(NCORES):
        sl = slice(c * BL, (c + 1) * BL)
        aohT = np.zeros((KPAD, BL), np.float32)
        aohT[:aoh.shape[1]] = aoh[sl].T
        in_maps.append({
            "ent": np.ascontiguousarray(ent[sl]),
            "aohT": aohT,
            "wembT": wembT,
            "bemb": bemb,
            "d1": D1,
            "e2": E2,
            "cwT": cwT,
            "c2wT": c2wT,
        })
    return in_maps


def run_on_hw(inputs, trace=False, trace_cores=None):
    nc = _get_nc()
    in_maps = _make_in_maps(inputs)
    br = run_bass_kernel_spmd(
        nc, in_maps, list(range(NCORES)), trace=trace,
        trace_cores=trace_cores,
    )
    maps = np.concatenate([r["attr_map"] for r in br.results], axis=0)
    feats = np.concatenate([r["attr_feature"] for r in br.results], axis=0)
    attr_map = maps.reshape(B, 1, H, W).astype(np.float32)
    attr_feature = feats.reshape(B, C, H, W).astype(np.float32)
    return (attr_map, attr_feature), br


def kernel(**inputs):
    (attr_map, attr_feature), _ = run_on_hw(inputs, trace=False)
    return attr_map, attr_feature


# revision 22
# speedup vs baseline: 1.1470x; 1.0345x over previous
"""Trainium2 Bass kernel for nn_AttributeBranch (MCB attribute pooling).

Algorithm note
--------------
In the reference, the "attr" operand of MCB is spatially constant
(emb[b, c] broadcast over H, W).  The MCB circular convolution with a
spatially-constant sketch therefore collapses to a per-batch circulant
matrix C_b[n, d] = sx_b[(n - d) mod 256], where sx_b is the count-sketch
of the embedding.  The whole network becomes

    hidden_b  = relu( (conv1_w @ C_b @ E2) @ y_b )        y_b = entity [256, 784]
    attr_map  = sigmoid(conv2_w @ hidden_b)
    attr_feat = attr_map * y_b

with E2[d, c] = s2[c] * (h2[c] == d) the dense count-sketch matrix.
Per batch we compute MT_b = (conv1_w @ C_b @ E2)^T  =  E2^T @ C_b^T @ conv1_w^T
via two tiny matmuls (the circulant tile is materialized by a strided DMA
from a periodic repeated-reversed copy of sx_b), then one
[32,256]x[256,784] matmul, conv2 + sigmoid, and a broadcast-multiply
against the entity tile.

Matmuls run in float32r (TF32-class, 1 cycle/row at N>=256, ~2e-4 rel err);
the final elementwise multiply and all accumulations stay fp32.

Sharding: pure data-parallel, batch 64 -> 8 cores x 8.
DMA queues: nc.sync (qSp HWDGE) carries loads, nc.scalar (qAct HWDGE)
carries stores + the circulant scratch write.
"""

import sys

for _p in ("/opt/trn_rl_repo",):
    if _p not in sys.path:
        sys.path.insert(0, _p)

from contextlib import ExitStack

import numpy as np

import concourse.bacc as bacc
import concourse.bass as bass
import concourse.mybir as mybir
import concourse.tile as tile
from concourse.ap import AP
from concourse.bass_utils import run_bass_kernel_spmd

NCORES = 8
B, C, H, W = 64, 256, 28, 28
HW = H * W
D = 256          # sketch dimension
BL = B // NCORES # batches per core
KPAD = 384       # attr dim 300 padded to 3x128

f32 = mybir.dt.float32
f32r = mybir.dt.float32r
AF = mybir.ActivationFunctionType


def _build_program():
    nc = bacc.Bacc("TRN2", target_bir_lowering=False, debug=False)

    # packed emb-path constants: [384, 521] =
    #   cols 0:256 wembT | 256:264 aohT | 264:520 d1 (rows 0:256) | 520 bemb
    PK = C + BL + D + 1
    ent_d = nc.dram_tensor("ent", [BL, C, HW], f32r, kind="ExternalInput")
    pack_d = nc.dram_tensor("pack", [KPAD, PK], f32r, kind="ExternalInput")
    e2_d = nc.dram_tensor("e2", [D, C], f32r, kind="ExternalInput")
    cwT_d = nc.dram_tensor("cwT", [C, 32], f32r, kind="ExternalInput")
    c2wT_d = nc.dram_tensor("c2wT", [32, 1], f32r, kind="ExternalInput")

    map_d = nc.dram_tensor("attr_map", [BL, HW], f32r, kind="ExternalOutput")
    af_d = nc.dram_tensor("attr_feature", [BL, C, HW], f32, kind="ExternalOutput")

    # Periodic repeated-reversed sketch rows, bounced through DRAM so the
    # circulant tiles can be materialized with a strided DMA read.  Row
    # layout: st_b[y] = sx_b[(-y) mod 256], repeated 129x; a read with
    # partition step +255 (= -1 mod 256) and free step +1 then yields
    # C[n, d] = sx[(n - d) mod 256].  (Negative DMA partition steps are
    # rejected by the BIR verifier, hence the +255 periodic trick.)
    NREP = 65
    SLEN = NREP * D
    rscr = nc.dram_tensor("rscr", [BL, SLEN], f32r)

    with tile.TileContext(nc) as tc, ExitStack() as ctx:
        const = ctx.enter_context(tc.tile_pool(name="const", bufs=1))
        work = ctx.enter_context(tc.tile_pool(name="work", bufs=4))
        entp = ctx.enter_context(tc.tile_pool(name="entp", bufs=16))
        circp = ctx.enter_context(tc.tile_pool(name="circp", bufs=6))
        afp = ctx.enter_context(tc.tile_pool(name="afp", bufs=4))
        psS = ctx.enter_context(tc.tile_pool(name="psS", bufs=2, space="PSUM"))
        psB = ctx.enter_context(tc.tile_pool(name="psB", bufs=3, space="PSUM"))

        def load_const(name, dram, nblk, cols, dt=f32r, eng=None):
            eng = eng or nc.sync
            ts = []
            for k in range(nblk):
                rows = min(128, dram.shape[0] - k * 128)
                t = const.tile([rows, cols], dt, tag=f"{name}{k}")
                eng.dma_start(t[:], dram.ap()[k * 128:k * 128 + rows, :])
                ts.append(t)
            return ts

        # emb-path constants first (they gate the circulant scratch write)
        pack = load_const("pack", pack_d, 3, C + BL + D + 1)
        wembT = [p[:, 0:C] for p in pack]
        aohT = [p[:, C:C + BL] for p in pack]
        d1 = [p[:, C + BL:C + BL + D] for p in pack[:2]]
        bemb = [p[:, C + BL + D:C + BL + D + 1].bitcast(f32) for p in pack[:2]]
        # second-group constants on the store queue (idle during prologue)
        e2 = load_const("e2", e2_d, 2, C, eng=nc.scalar)
        cwT = load_const("cwT", cwT_d, 2, 32, eng=nc.scalar)
        c2wT = load_const("c2wT", c2wT_d, 1, 1, eng=nc.scalar)[0]
        ones_f = const.tile([1, 128], f32, tag="ones_f")
        nc.vector.memset(ones_f[:], 1.0)
        ones = const.tile([1, 128], f32r, tag="ones")
        nc.vector.tensor_copy(ones[:], ones_f[:])

        # ---- embT[c, b] = (W_emb @ aoh^T + b_emb) ----------------------
        embT = []
        for mb in range(2):
            pe = psS.tile([128, BL], f32, tag="ps_small")
            for k in range(3):
                nc.tensor.matmul(pe[:], wembT[k][:, mb * 128:(mb + 1) * 128],
                                 aohT[k][:], start=(k == 0), stop=(k == 2))
            t = work.tile([128, BL], f32r, tag=f"embT{mb}")
            nc.vector.tensor_scalar_add(t[:], pe[:], bemb[mb])
            embT.append(t)

        # ---- sx[b, d] = (emb @ D1) ------------------------------------
        psx = psS.tile([BL, D], f32, tag="ps_small")
        for kb in range(2):
            nc.tensor.matmul(psx[:], embT[kb][:], d1[kb][:],
                             start=(kb == 0), stop=(kb == 1))
        # crow[b, y] = sx[b, (-y) mod 256], read straight out of PSUM
        crow = work.tile([BL, D], f32r, tag="crow")
        nc.vector.tensor_copy(crow[:, 0:1], psx[:, 0:1])
        nc.vector.tensor_copy(crow[:, 1:D], psx[:, 1:D][:, ::-1])
        # Stage the rows in DRAM, then replicate DRAM->DRAM per batch.
        # (An SBUF-sourced broadcast write re-reads each 1KB partition row
        # 129x through one SBUF port -- ~25us stall.  DRAM-sourced
        # replication runs at queue bandwidth.)
        crow_d = nc.dram_tensor("crow_d", [BL, D], f32r)
        cw_st = nc.scalar.dma_start(crow_d.ap()[:], crow[:])
        reps = []
        for b in range(BL):
            eng = nc.scalar if b % 2 == 0 else nc.sync
            r = eng.dma_start(
                AP(rscr, b * SLEN, [[D, NREP], [1, D]]),
                AP(crow_d, b * D, [[0, NREP], [1, D]]))
            tile.add_dep_helper(r.ins, cw_st.ins, reason="crow_d RAW")
            reps.append(r)

        # ---- per-batch pipeline ---------------------------------------
        NSPLIT = [(0, 512), (512, HW - 512)]
        for b in range(BL):
            # circulant tiles C[n, d] = sx[(n - d) % 256] = st_b[A + 255*n + d]
            # with A == -n0 (mod 256) per 64-partition chunk
            circ = []
            circ_lds = []
            for nb in range(2):
                t = circp.tile([128, D], f32r, tag="circ")
                for half in range(2):
                    n0 = 128 * nb + 64 * half
                    base = (-n0) % 256
                    src = AP(rscr, b * SLEN + base, [[255, 64], [1, D]])
                    ld = nc.sync.dma_start(t[64 * half:64 * half + 64, :], src)
                    tile.add_dep_helper(ld.ins, reps[b].ins, reason="rscr RAW")
                    circ_lds.append(ld)
                circ.append(t)

            ent_t = []
            for cb in range(2):
                t = entp.tile([128, HW], f32r, tag="ent")
                eld = nc.sync.dma_start(
                    t[:], ent_d.ap()[b, cb * 128:(cb + 1) * 128, :])
                if b >= 2:
                    # pace the prefetch: keep the queue FIFOs from running
                    # ahead of the latency-critical circulant chain
                    tile.add_dep_helper(eld.ins, circ_lds[0].ins,
                                        reason="prefetch pacing", sync=False)
                ent_t.append(t)

            # V[d, o] = sum_n C[n, d] * conv1_w[o, n]
            v = []
            for db in range(2):
                pv = psS.tile([128, 32], f32, tag="ps_small")
                for nb in range(2):
                    nc.tensor.matmul(pv[:], circ[nb][:, db * 128:(db + 1) * 128],
                                     cwT[nb][:], start=(nb == 0), stop=(nb == 1))
                t = work.tile([128, 32], f32r, tag="v")
                nc.vector.tensor_copy(t[:], pv[:])
                v.append(t)

            # MT[c, o] = sum_d E2[d, c] * V[d, o]
            mt = []
            for cb in range(2):
                pm = psS.tile([128, 32], f32, tag="ps_small")
                for db in range(2):
                    nc.tensor.matmul(pm[:], e2[db][:, cb * 128:(cb + 1) * 128],
                                     v[db][:], start=(db == 0), stop=(db == 1))
                t = work.tile([128, 32], f32r, tag="mt")
                nc.vector.tensor_copy(t[:], pm[:])
                mt.append(t)

            # hidden[o, hw] = relu(sum_c MT[c, o] * ent[c, hw])
            ph = psB.tile([32, HW], f32, tag="ps_big")
            for n0, nn in NSPLIT:
                for cb in range(2):
                    nc.tensor.matmul(ph[:, n0:n0 + nn], mt[cb][:],
                                     ent_t[cb][:, n0:n0 + nn],
                                     start=(cb == 0), stop=(cb == 1))
            hid = work.tile([32, HW], f32r, tag="hid")
            nc.scalar.activation(hid[:], ph[:], AF.Relu)

            # attr_map[hw] = sigmoid(sum_o conv2_w[0, o] * hidden[o, hw])
            pmap = psB.tile([1, HW], f32, tag="ps_big")
            for n0, nn in NSPLIT:
                nc.tensor.matmul(pmap[:, n0:n0 + nn], c2wT[:],
                                 hid[:, n0:n0 + nn], start=True, stop=True)
            map_t = work.tile([1, HW], f32r, tag="map")
            nc.scalar.activation(map_t[:], pmap[:], AF.Sigmoid)
            nc.scalar.dma_start(map_d.ap()[b:b + 1, :], map_t[:])

            # broadcast map across 128 partitions on GPSIMD (PE stays free)
            bc = work.tile([128, HW], f32, tag="bc")
            nc.gpsimd.partition_broadcast(bc[:], map_t[:].bitcast(f32))

            for cb in range(2):
                af = afp.tile([128, HW], f32, tag="af")
                nc.vector.tensor_tensor(af[:], ent_t[cb][:].bitcast(f32), bc[:],
                                        op=mybir.AluOpType.mult)
                nc.scalar.dma_start(
                    af_d.ap()[b, cb * 128:(cb + 1) * 128, :], af[:])

    nc.compile()
    return nc


_NC = None


def _get_nc():
    global _NC
    if _NC is None:
        _NC = _build_program()
    return _NC


def _make_in_maps(inputs):
    ent = np.ascontiguousarray(inputs["entity_feature"], np.float32).reshape(B, C, HW)
    aoh = np.asarray(inputs["attr_one_hot"], np.float32)
    Wemb = np.asarray(inputs["W_emb"], np.float32)
    bemb = np.asarray(inputs["b_emb"], np.float32).reshape(C, 1)
    h1 = np.asarray(inputs["h1"]).astype(np.int64)
    s1 = np.asarray(inputs["s1"], np.float32)
    h2 = np.asarray(inputs["h2"]).astype(np.int64)
    s2 = np.asarray(inputs["s2"], np.float32)
    cw = np.asarray(inputs["conv1_w"], np.float32)
    c2w = np.asarray(inputs["conv2_w"], np.float32)

    # dense count-sketch matrices (pure re-indexing of h/s inputs)
    D1 = np.zeros((C, D), np.float32)
    D1[np.arange(C), h1] = s1
    E2 = np.zeros((D, C), np.float32)
    E2[h2, np.arange(C)] = s2

    cwT = np.ascontiguousarray(cw.T)
    c2wT = np.ascontiguousarray(c2w.T)

    in_maps = []
    for c in range(NCORES):
        sl = slice(c * BL, (c + 1) * BL)
        pack = np.zeros((KPAD, C + BL + D + 1), np.float32)
        pack[:Wemb.shape[1], 0:C] = Wemb.T
        pack[:aoh.shape[1], C:C + BL] = aoh[sl].T
        pack[:C, C + BL:C + BL + D] = D1
        pack[:C, C + BL + D] = bemb[:, 0]
        in_maps.append({
            "ent": np.ascontiguousarray(ent[sl]),
            "pack": pack,
            "e2": E2,
            "cwT": cwT,
            "c2wT": c2wT,
        })
    return in_maps


def run_on_hw(inputs, trace=False, trace_cores=None):
    nc = _get_nc()
    in_maps = _make_in_maps(inputs)
    br = run_bass_kernel_spmd(
        nc, in_maps, list(range(NCORES)), trace=trace,
        trace_cores=trace_cores,
    )
    maps = np.concatenate([r["attr_map"] for r in br.results], axis=0)
    feats = np.concatenate([r["attr_feature"] for r in br.results], axis=0)
    attr_map = maps.reshape(B, 1, H, W).astype(np.float32)
    attr_feature = feats.reshape(B, C, H, W).astype(np.float32)
    return (attr_map, attr_feature), br


def kernel(**inputs):
    (attr_map, attr_feature), _ = run_on_hw(inputs, trace=False)
    return attr_map, attr_feature


# revision 23
# speedup vs baseline: 1.2238x; 1.0669x over previous
"""Trainium2 Bass kernel for nn_AttributeBranch (MCB attribute pooling).

Algorithm note
--------------
In the reference, the "attr" operand of MCB is spatially constant
(emb[b, c] broadcast over H, W).  The MCB circular convolution with a
spatially-constant sketch therefore collapses to a per-batch circulant
matrix C_b[n, d] = sx_b[(n - d) mod 256], where sx_b is the count-sketch
of the embedding.  The whole network becomes

    hidden_b  = relu( (conv1_w @ C_b @ E2) @ y_b )        y_b = entity [256, 784]
    attr_map  = sigmoid(conv2_w @ hidden_b)
    attr_feat = attr_map * y_b

with E2[d, c] = s2[c] * (h2[c] == d) the dense count-sketch matrix.
Per batch we compute MT_b = (conv1_w @ C_b @ E2)^T  =  E2^T @ C_b^T @ conv1_w^T
via two tiny matmuls (the circulant tile is materialized by a strided DMA
from a periodic repeated-reversed copy of sx_b), then one
[32,256]x[256,784] matmul, conv2 + sigmoid, and a broadcast-multiply
against the entity tile.

Matmuls run in float32r (TF32-class, 1 cycle/row at N>=256, ~2e-4 rel err);
the final elementwise multiply and all accumulations stay fp32.

Sharding: pure data-parallel, batch 64 -> 8 cores x 8.
DMA queues: nc.sync (qSp HWDGE) carries loads, nc.scalar (qAct HWDGE)
carries stores + the circulant scratch write.
"""

import sys

for _p in ("/opt/trn_rl_repo",):
    if _p not in sys.path:
        sys.path.insert(0, _p)

from contextlib import ExitStack

import numpy as np

import concourse.bacc as bacc
import concourse.bass as bass
import concourse.mybir as mybir
import concourse.tile as tile
from concourse.ap import AP
from concourse.bass_utils import run_bass_kernel_spmd

NCORES = 8
B, C, H, W = 64, 256, 28, 28
HW = H * W
D = 256          # sketch dimension
BL = B // NCORES # batches per core
KPAD = 384       # attr dim 300 padded to 3x128

f32 = mybir.dt.float32
f32r = mybir.dt.float32r
AF = mybir.ActivationFunctionType


def _build_program():
    nc = bacc.Bacc("TRN2", target_bir_lowering=False, debug=False)

    # packed emb-path constants: [384, 521] =
    #   cols 0:256 wembT | 256:264 aohT | 264:520 d1 (rows 0:256) | 520 bemb
    PK = C + BL + D + 1
    ent_d = nc.dram_tensor("ent", [BL, C, HW], f32r, kind="ExternalInput")
    pack_d = nc.dram_tensor("pack", [KPAD, PK], f32r, kind="ExternalInput")
    e2_d = nc.dram_tensor("e2", [D, C], f32r, kind="ExternalInput")
    cwT_d = nc.dram_tensor("cwT", [C, 32], f32r, kind="ExternalInput")
    c2wT_d = nc.dram_tensor("c2wT", [32, 1], f32r, kind="ExternalInput")

    map_d = nc.dram_tensor("attr_map", [BL, HW], f32r, kind="ExternalOutput")
    af_d = nc.dram_tensor("attr_feature", [BL, C, HW], f32, kind="ExternalOutput")

    # Periodic repeated-reversed sketch rows, bounced through DRAM so the
    # circulant tiles can be materialized with a strided DMA read.  Row
    # layout: st_b[y] = sx_b[(-y) mod 256], repeated 129x; a read with
    # partition step +255 (= -1 mod 256) and free step +1 then yields
    # C[n, d] = sx[(n - d) mod 256].  (Negative DMA partition steps are
    # rejected by the BIR verifier, hence the +255 periodic trick.)
    NREP = 65
    SLEN = NREP * D
    rscr = nc.dram_tensor("rscr", [BL, SLEN], f32r)

    with tile.TileContext(nc) as tc, ExitStack() as ctx:
        const = ctx.enter_context(tc.tile_pool(name="const", bufs=1))
        work = ctx.enter_context(tc.tile_pool(name="work", bufs=4))
        entp = ctx.enter_context(tc.tile_pool(name="entp", bufs=16))
        circp = ctx.enter_context(tc.tile_pool(name="circp", bufs=6))
        afp = ctx.enter_context(tc.tile_pool(name="afp", bufs=4))
        psS = ctx.enter_context(tc.tile_pool(name="psS", bufs=2, space="PSUM"))
        psB = ctx.enter_context(tc.tile_pool(name="psB", bufs=3, space="PSUM"))

        def load_const(name, dram, nblk, cols, dt=f32r, eng=None):
            eng = eng or nc.sync
            ts = []
            for k in range(nblk):
                rows = min(128, dram.shape[0] - k * 128)
                t = const.tile([rows, cols], dt, tag=f"{name}{k}")
                eng.dma_start(t[:], dram.ap()[k * 128:k * 128 + rows, :])
                ts.append(t)
            return ts

        # emb-path constants first (they gate the circulant scratch write)
        pack = load_const("pack", pack_d, 3, C + BL + D + 1)
        wembT = [p[:, 0:C] for p in pack]
        aohT = [p[:, C:C + BL] for p in pack]
        d1 = [p[:, C + BL:C + BL + D] for p in pack[:2]]
        bemb = [p[:, C + BL + D:C + BL + D + 1].bitcast(f32) for p in pack[:2]]
        # second-group constants on the store queue (idle during prologue)
        e2 = load_const("e2", e2_d, 2, C, eng=nc.scalar)
        cwT = load_const("cwT", cwT_d, 2, 32, eng=nc.scalar)
        c2wT = load_const("c2wT", c2wT_d, 1, 1, eng=nc.scalar)[0]

        # ---- embT[c, b] = (W_emb @ aoh^T + b_emb) ----------------------
        embT = []
        for mb in range(2):
            pe = psS.tile([128, BL], f32, tag="ps_small")
            for k in range(3):
                nc.tensor.matmul(pe[:], wembT[k][:, mb * 128:(mb + 1) * 128],
                                 aohT[k][:], start=(k == 0), stop=(k == 2))
            t = work.tile([128, BL], f32r, tag=f"embT{mb}")
            nc.vector.tensor_scalar_add(t[:], pe[:], bemb[mb])
            embT.append(t)

        # ---- sx[b, d] = (emb @ D1) ------------------------------------
        psx = psS.tile([BL, D], f32, tag="ps_small")
        for kb in range(2):
            nc.tensor.matmul(psx[:], embT[kb][:], d1[kb][:],
                             start=(kb == 0), stop=(kb == 1))
        # crow[b, y] = sx[b, (-y) mod 256], read straight out of PSUM
        crow = work.tile([BL, D], f32r, tag="crow")
        nc.vector.tensor_copy(crow[:, 0:1], psx[:, 0:1])
        nc.vector.tensor_copy(crow[:, 1:D], psx[:, 1:D][:, ::-1])
        # Stage the rows in DRAM, then replicate DRAM->DRAM per batch.
        # (An SBUF-sourced broadcast write re-reads each 1KB partition row
        # 129x through one SBUF port -- ~25us stall.  DRAM-sourced
        # replication runs at queue bandwidth.)
        crow_d = nc.dram_tensor("crow_d", [BL, D], f32r)
        cw_st = nc.scalar.dma_start(crow_d.ap()[:], crow[:])
        reps = []
        for b in range(BL):
            eng = nc.scalar if b % 2 == 0 else nc.sync
            r = eng.dma_start(
                AP(rscr, b * SLEN, [[D, NREP], [1, D]]),
                AP(crow_d, b * D, [[0, NREP], [1, D]]))
            tile.add_dep_helper(r.ins, cw_st.ins, reason="crow_d RAW")
            reps.append(r)

        # ---- per-batch pipeline ---------------------------------------
        NSPLIT = [(0, 512), (512, HW - 512)]
        for b in range(BL):
            # circulant tiles C[n, d] = sx[(n - d) % 256] = st_b[A + 255*n + d]
            # with A == -n0 (mod 256) per 64-partition chunk
            circ = []
            circ_lds = []
            for nb in range(2):
                t = circp.tile([128, D], f32r, tag="circ")
                for half in range(2):
                    n0 = 128 * nb + 64 * half
                    base = (-n0) % 256
                    src = AP(rscr, b * SLEN + base, [[255, 64], [1, D]])
                    ld = nc.sync.dma_start(t[64 * half:64 * half + 64, :], src)
                    tile.add_dep_helper(ld.ins, reps[b].ins, reason="rscr RAW")
                    circ_lds.append(ld)
                circ.append(t)

            ent_t = []
            for cb in range(2):
                t = entp.tile([128, HW], f32r, tag="ent")
                eld = nc.sync.dma_start(
                    t[:], ent_d.ap()[b, cb * 128:(cb + 1) * 128, :])
                if b >= 2:
                    # pace the prefetch: keep the queue FIFOs from running
                    # ahead of the latency-critical circulant chain
                    tile.add_dep_helper(eld.ins, circ_lds[0].ins,
                                        reason="prefetch pacing", sync=False)
                ent_t.append(t)

            # V[d, o] = sum_n C[n, d] * conv1_w[o, n]
            v = []
            for db in range(2):
                pv = psS.tile([128, 32], f32, tag="ps_small")
                for nb in range(2):
                    nc.tensor.matmul(pv[:], circ[nb][:, db * 128:(db + 1) * 128],
                                     cwT[nb][:], start=(nb == 0), stop=(nb == 1))
                t = work.tile([128, 32], f32r, tag="v")
                nc.vector.tensor_copy(t[:], pv[:])
                v.append(t)

            # MT[c, o] = sum_d E2[d, c] * V[d, o]
            mt = []
            for cb in range(2):
                pm = psS.tile([128, 32], f32, tag="ps_small")
                for db in range(2):
                    nc.tensor.matmul(pm[:], e2[db][:, cb * 128:(cb + 1) * 128],
                                     v[db][:], start=(db == 0), stop=(db == 1))
                t = work.tile([128, 32], f32r, tag="mt")
                nc.vector.tensor_copy(t[:], pm[:])
                mt.append(t)

            # hidden[o, hw] = relu(sum_c MT[c, o] * ent[c, hw])
            ph = psB.tile([32, HW], f32, tag="ps_big")
            for n0, nn in NSPLIT:
                for cb in range(2):
                    nc.tensor.matmul(ph[:, n0:n0 + nn], mt[cb][:],
                                     ent_t[cb][:, n0:n0 + nn],
                                     start=(cb == 0), stop=(cb == 1))
            hid = work.tile([32, HW], f32r, tag="hid")
            nc.scalar.activation(hid[:], ph[:], AF.Relu)

            # attr_map[hw] = sigmoid(sum_o conv2_w[0, o] * hidden[o, hw])
            pmap = psB.tile([1, HW], f32, tag="ps_big")
            for n0, nn in NSPLIT:
                nc.tensor.matmul(pmap[:, n0:n0 + nn], c2wT[:],
                                 hid[:, n0:n0 + nn], start=True, stop=True)
            map_t = work.tile([1, HW], f32r, tag="map")
            nc.scalar.activation(map_t[:], pmap[:], AF.Sigmoid)
            nc.scalar.dma_start(map_d.ap()[b:b + 1, :], map_t[:])

            # broadcast map across 128 partitions on GPSIMD (PE stays free)
            bc = work.tile([128, HW], f32, tag="bc")
            nc.gpsimd.partition_broadcast(bc[:], map_t[:].bitcast(f32))

            for cb in range(2):
                af = afp.tile([128, HW], f32, tag="af")
                nc.vector.tensor_tensor(af[:], ent_t[cb][:].bitcast(f32), bc[:],
                                        op=mybir.AluOpType.mult)
                nc.scalar.dma_start(
                    af_d.ap()[b, cb * 128:(cb + 1) * 128, :], af[:])

    nc.compile()
    return nc


_NC = None


def _get_nc():
    global _NC
    if _NC is None:
        _NC = _build_program()
    return _NC


def _make_in_maps(inputs):
    ent = np.ascontiguousarray(inputs["entity_feature"], np.float32).reshape(B, C, HW)
    aoh = np.asarray(inputs["attr_one_hot"], np.float32)
    Wemb = np.asarray(inputs["W_emb"], np.float32)
    bemb = np.asarray(inputs["b_emb"], np.float32).reshape(C, 1)
    h1 = np.asarray(inputs["h1"]).astype(np.int64)
    s1 = np.asarray(inputs["s1"], np.float32)
    h2 = np.asarray(inputs["h2"]).astype(np.int64)
    s2 = np.asarray(inputs["s2"], np.float32)
    cw = np.asarray(inputs["conv1_w"], np.float32)
    c2w = np.asarray(inputs["conv2_w"], np.float32)

    # dense count-sketch matrices (pure re-indexing of h/s inputs)
    D1 = np.zeros((C, D), np.float32)
    D1[np.arange(C), h1] = s1
    E2 = np.zeros((D, C), np.float32)
    E2[h2, np.arange(C)] = s2

    cwT = np.ascontiguousarray(cw.T)
    c2wT = np.ascontiguousarray(c2w.T)

    in_maps = []
    for c in range(NCORES):
        sl = slice(c * BL, (c + 1) * BL)
        pack = np.zeros((KPAD, C + BL + D + 1), np.float32)
        pack[:Wemb.shape[1], 0:C] = Wemb.T
        pack[:aoh.shape[1], C:C + BL] = aoh[sl].T
        pack[:C, C + BL:C + BL + D] = D1
        pack[:C, C + BL + D] = bemb[:, 0]
        in_maps.append({
            "ent": np.ascontiguousarray(ent[sl]),
            "pack": pack,
            "e2": E2,
            "cwT": cwT,
            "c2wT": c2wT,
        })
    return in_maps


def run_on_hw(inputs, trace=False, trace_cores=None):
    nc = _get_nc()
    in_maps = _make_in_maps(inputs)
    br = run_bass_kernel_spmd(
        nc, in_maps, list(range(NCORES)), trace=trace,
        trace_cores=trace_cores,
    )
    maps = np.concatenate([r["attr_map"] for r in br.results], axis=0)
    feats = np.concatenate([r["attr_feature"] for r in br.results], axis=0)
    attr_map = maps.reshape(B, 1, H, W).astype(np.float32)
    attr_feature = feats.reshape(B, C, H, W).astype(np.float32)
    return (attr_map, attr_feature), br


def kernel(**inputs):
    (attr_map, attr_feature), _ = run_on_hw(inputs, trace=False)
    return attr_map, attr_feature
